# revision 1
# baseline (speedup 1.0000x reference)
"""GAT (2-layer, heads=1) on 8 Trainium2 NeuronCores.

Strategy (1D node partition, per the sharding hint):
  - Nodes are split into 8 chunks of NL; core c owns dst-chunk c.
  - Layer tables h^T/alpha_src are computed per-chunk (feature-major via
    PE matmuls on x^T), AllGathered, and kept in SBUF as a per-partition-group
    table: partition group g (16 partitions) holds (h|alpha_src) of chunk g,
    packed two bf16 per fp32 word.
  - Edges are bucketed on the host by (dst-core, src-chunk, dst-block) and
    sorted by dst. Each edge tile covers one block of NB dst nodes for all
    8 src-chunk groups at once; per-edge gathers run on GPSIMD (ap_gather),
    per-edge math on DVE/ACT, and segment-sums via masked tensor_tensor_scan
    (state = mask*state + w, mask=0 at segment starts) + a boundary gather at
    each node's last edge. Group partials are combined with a matmul against
    a 0/1 selection matrix; softmax normalization divides at the node level.
  - Softmax max-subtraction is skipped: attention logits here are O(1), and
    alpha = exp(e)/sum(exp(e)) is shift-invariant.

Host preprocessing only reorders/buckets edge indices (structure), never
touches float data dependent on device results.
"""

import math
import numpy as np

import ml_dtypes

from concourse import bass, bacc, mybir
import concourse.tile as tile

F32 = mybir.dt.float32
BF16 = mybir.dt.bfloat16
I16 = mybir.dt.int16

NEG_SLOPE = 0.2
# fp32 -29952.0 == 0xC6EA0000: low bf16 lane (h) = 0, high bf16 lane
# (alpha_src) = -29952 -> exp(lrelu(...)) == 0, so sentinel edges vanish.
SENTINEL_PACKED = -29952.0

FULL_CFG = dict(
    NCORES=8, N=100000, F=512, H=16,
    NL=12500, NB=500, NT=25, CH=500, NCH=25,
)


# ---------------------------------------------------------------- host prep

def _round_up(x, m):
    return (x + m - 1) // m * m


def host_prep(edge_index, cfg):
    """Bucket and sort edges; build device index/mask streams.

    Returns (T_e, per_core) where per_core[c] has 'idxs' [128, NT*IW] i16 and
    'mask' [128, NT*T_e] bf16.
    """
    NC, N, NL = cfg["NCORES"], cfg["N"], cfg["NL"]
    NB, NT = cfg["NB"], cfg["NT"]
    G = 8

    # Self-loops are NOT added here: their contribution is computed
    # analytically at the node level on-device (no gather needed).
    src = np.asarray(edge_index[0], dtype=np.int64)
    dst = np.asarray(edge_index[1], dtype=np.int64)

    core = dst // NL
    grp = src // NL
    blk = (dst % NL) // NB
    order = np.lexsort((src, dst, grp, core))
    src, dst, core, grp, blk = (a[order] for a in (src, dst, core, grp, blk))

    bucket = (core * G + grp) * NT + blk
    nbuck = NC * G * NT
    counts = np.bincount(bucket, minlength=nbuck)
    T_e = _round_up(int(counts.max()) + 16, 128)
    assert T_e <= 32767

    starts = np.zeros(nbuck, dtype=np.int64)
    starts[1:] = np.cumsum(counts)[:-1]
    pos = np.arange(src.size) - starts[bucket]

    prev_same = np.zeros(src.size, dtype=bool)
    prev_same[1:] = (bucket[1:] == bucket[:-1]) & (dst[1:] == dst[:-1])
    is_last = np.ones(src.size, dtype=bool)
    is_last[:-1] = ~((bucket[1:] == bucket[:-1]) & (dst[1:] == dst[:-1]))

    srcl = np.full((NC, G, NT, T_e), NL, dtype=np.int16)
    dstl = np.full((NC, G, NT, T_e), NB, dtype=np.int16)
    maskv = np.zeros((NC, G, NT, T_e), dtype=np.float32)
    bnd = np.full((NC, G, NT, 512), T_e - 1, dtype=np.int16)

    c_, g_, b_ = core, grp, blk
    srcl[c_, g_, b_, pos] = (src % NL).astype(np.int16)
    dstl[c_, g_, b_, pos] = ((dst % NL) % NB).astype(np.int16)
    maskv[c_[prev_same], g_[prev_same], b_[prev_same], pos[prev_same]] = 1.0
    bnd[c_[is_last], g_[is_last], b_[is_last],
        ((dst % NL) % NB)[is_last]] = pos[is_last].astype(np.int16)

    def wrap(a, w):
        # [NC, G, NT, w] -> [NC, 128, NT, w//16]; w16[c, 16g+p, t, s] = a[c, g, t, s*16+p]
        n = a.shape[-1]
        return (a.reshape(NC, G, NT, n // 16, 16)
                 .transpose(0, 1, 4, 2, 3)
                 .reshape(NC, 128, NT, n // 16))

    src_w, dst_w, bnd_w = wrap(srcl, T_e), wrap(dstl, T_e), wrap(bnd, 512)
    idxs = np.concatenate([src_w, dst_w, bnd_w], axis=3)  # [NC,128,NT,IW]
    IW = idxs.shape[3]
    idxs = np.ascontiguousarray(idxs.reshape(NC, 128, NT * IW))

    mask_rep = np.repeat(maskv, 16, axis=1)  # [NC, 128, NT, T_e]
    mask_rep = np.ascontiguousarray(
        mask_rep.reshape(NC, 128, NT * T_e).astype(ml_dtypes.bfloat16))

    per_core = [{"idxs": idxs[c], "mask": mask_rep[c]} for c in range(NC)]
    return T_e, per_core


# ------------------------------------------------------------- device build

def build_nc(cfg, T_e, max_waits=2, ctrl_max_waits=1, split=True):
    NC, N, F, H = cfg["NCORES"], cfg["N"], cfg["F"], cfg["H"]
    NL, NB, NT = cfg["NL"], cfg["NB"], cfg["NT"]
    CH, NCH = cfg["CH"], cfg["NCH"]
    KB = F // 128
    NLP = NL + 16               # table width incl. sentinel
    NBW = NB + 16               # per-tile alpha_dst table width
    T16 = T_e // 16
    IW = 2 * T16 + 512 // 16
    NBLK = math.ceil(NL / 128)
    NL2 = NBLK * 128
    rg = [list(range(NC))]

    nc = bacc.Bacc("TRN2", target_bir_lowering=False)

    xt = nc.declare_dram_parameter("xt", [F, NL], F32, isOutput=False)
    w1 = nc.declare_dram_parameter("w1", [F, H], F32, isOutput=False)
    w2 = nc.declare_dram_parameter("w2", [H, H], F32, isOutput=False)
    a1rep = nc.declare_dram_parameter("a1rep", [H, 16], F32, isOutput=False)
    ad1rep = nc.declare_dram_parameter("ad1rep", [H, 16], F32, isOutput=False)
    a2rep = nc.declare_dram_parameter("a2rep", [H, 16], F32, isOutput=False)
    ad2rep = nc.declare_dram_parameter("ad2rep", [H, 16], F32, isOutput=False)
    b1p = nc.declare_dram_parameter("b1p", [H, 1], F32, isOutput=False)
    b2p = nc.declare_dram_parameter("b2p", [H, 1], F32, isOutput=False)
    selp = nc.declare_dram_parameter("selp", [128, 16], F32, isOutput=False)
    identp = nc.declare_dram_parameter("identp", [16, 16], F32, isOutput=False)
    idxsp = nc.declare_dram_parameter("idxs", [128, NT * IW], I16, isOutput=False)
    maskp = nc.declare_dram_parameter("mask", [128, NT * T_e], BF16, isOutput=False)
    outp = nc.declare_dram_parameter("out", [128, NBLK * H], F32, isOutput=True)

    ag_in = [nc.dram_tensor(f"ag_in{l}", [16, NL], F32) for l in (1, 2)]
    ag_out = [nc.dram_tensor(f"ag_out{l}", [128, NL], F32, addr_space="Shared")
              for l in (1, 2)]
    ad_stage = [nc.dram_tensor(f"ad_stage{l}", [1, NL + NBW], F32) for l in (1, 2)]
    pself_stage = [nc.dram_tensor(f"pself{l}", [1, NL], F32) for l in (1, 2)]
    wself_stage = [nc.dram_tensor(f"wself{l}", [16, NL], F32) for l in (1, 2)]

    with tile.TileContext(nc, num_cores=NC) as tc:
        with tc.tile_pool(name="const", bufs=1) as cpool:
            w1t = cpool.tile([128, KB, H], F32)
            nc.sync.dma_start(out=w1t[:], in_=w1[:].rearrange("(b p) h -> p b h", p=128))
            w2t = cpool.tile([16, H], F32)
            nc.sync.dma_start(out=w2t[:], in_=w2[:])
            a1t = cpool.tile([16, 16], F32)
            nc.sync.dma_start(out=a1t[:], in_=a1rep[:])
            ad1t = cpool.tile([16, 16], F32)
            nc.sync.dma_start(out=ad1t[:], in_=ad1rep[:])
            a2t = cpool.tile([16, 16], F32)
            nc.sync.dma_start(out=a2t[:], in_=a2rep[:])
            ad2t = cpool.tile([16, 16], F32)
            nc.sync.dma_start(out=ad2t[:], in_=ad2rep[:])
            b1t = cpool.tile([16, 1], F32)
            nc.sync.dma_start(out=b1t[:], in_=b1p[:])
            b2t = cpool.tile([16, 1], F32)
            nc.sync.dma_start(out=b2t[:], in_=b2p[:])
            selt = cpool.tile([128, 16], F32)
            nc.sync.dma_start(out=selt[:], in_=selp[:])
            idt = cpool.tile([16, 16], F32)
            nc.sync.dma_start(out=idt[:], in_=identp[:])
            zero16 = cpool.tile([1, NBW], F32)
            nc.vector.memset(zero16[:], 0.0)

            # out1T and out2T share one slot: all out1T reads (phase A2)
            # complete before any out2T write (layer-2 edge phase).
            out1T = cpool.tile([16, NL2], F32, tag="outT")
            out2T = cpool.tile([16, NL2], F32, tag="outT")

            # ---------------- phase A (layer 1): tables from x^T ----------
            with (
                tc.tile_pool(name="pa", bufs=2) as pa,
                tc.tile_pool(name="pap", bufs=2, space="PSUM") as pap,
            ):
                for c in range(NCH):
                    sl = slice(c * CH, (c + 1) * CH)
                    xt_t = pa.tile([128, KB, CH], F32, tag="xt")
                    nc.sync.dma_start(
                        out=xt_t[:],
                        in_=xt[:, sl].rearrange("(b p) n -> p b n", p=128))
                    ph = pap.tile([16, CH], F32, tag="ph")
                    for b in range(KB):
                        nc.tensor.matmul(ph[:], lhsT=w1t[:, b, :], rhs=xt_t[:, b, :],
                                         start=(b == 0), stop=(b == KB - 1))
                    hch = pa.tile([16, CH], F32, tag="hch")
                    nc.vector.tensor_copy(hch[:], ph[:])
                    pas = pap.tile([16, CH], F32, tag="pas")
                    nc.tensor.matmul(pas[:], lhsT=a1t[:], rhs=hch[:], start=True, stop=True)
                    pad_ = pap.tile([16, CH], F32, tag="pad")
                    nc.tensor.matmul(pad_[:], lhsT=ad1t[:], rhs=hch[:], start=True, stop=True)
                    packed = pa.tile([16, CH], F32, tag="packed")
                    pb = packed[:].bitcast(BF16)
                    nc.vector.tensor_copy(pb[:, 0::2], hch[:])
                    nc.vector.tensor_copy(pb[:, 1::2], pas[:])
                    nc.sync.dma_start(out=ag_in[0][:, sl], in_=packed[:])
                    adfull = pa.tile([16, CH], F32, tag="adfull")
                    nc.vector.tensor_copy(adfull[:], pad_[:])
                    adrow = pa.tile([1, CH], F32, tag="adrow")
                    nc.vector.tensor_copy(adrow[:], adfull[0:1, :])
                    nc.sync.dma_start(out=ad_stage[0][0:1, sl], in_=adrow[:])
                    # analytic self-loop contribution for this chunk
                    tself = pa.tile([16, CH], F32, tag="tself")
                    nc.vector.tensor_add(tself[:], pas[:], adfull[:])
                    nc.vector.scalar_tensor_tensor(
                        tself[:], tself[:], NEG_SLOPE, tself[:],
                        op0=mybir.AluOpType.mult, op1=mybir.AluOpType.max)
                    pselfc = pa.tile([16, CH], F32, tag="pselfc")
                    nc.scalar.activation(pselfc[:], tself[:],
                                         mybir.ActivationFunctionType.Exp)
                    wselfc = pa.tile([16, CH], F32, tag="wselfc")
                    nc.vector.tensor_mul(wselfc[:], pselfc[:], hch[:])
                    nc.sync.dma_start(out=wself_stage[0][:, sl], in_=wselfc[:])
                    nc.sync.dma_start(out=pself_stage[0][0:1, sl], in_=pselfc[0:1, :])
                nc.sync.dma_start(out=ad_stage[0][0:1, NL:], in_=zero16[:])

            def run_layer(l, writer):
                nc.gpsimd.collective_compute(
                    "AllGather", mybir.AluOpType.bypass, replica_groups=rg,
                    ins=[ag_in[l][:]], outs=[ag_out[l][:]])
                with tc.tile_pool(name=f"tab{l}", bufs=1) as tabp:
                    table = tabp.tile([128, NLP], F32, tag="table")
                    nc.sync.dma_start(out=table[:, :NL], in_=ag_out[l][:])
                    nc.vector.memset(table[:, NL:], SENTINEL_PACKED)
                    with (
                        tc.tile_pool(name=f"ed{l}", bufs=2) as ed,
                        tc.tile_pool(name=f"eo{l}", bufs=1) as eo,
                        tc.tile_pool(name=f"edp{l}", bufs=2, space="PSUM") as edp,
                    ):
                        for t in range(NT):
                            it = ed.tile([128, IW], I16, tag="it")
                            nc.sync.dma_start(out=it[:], in_=idxsp[:, t * IW:(t + 1) * IW])
                            mt = eo.tile([128, T_e], BF16, tag="mt")
                            nc.sync.dma_start(out=mt[:], in_=maskp[:, t * T_e:(t + 1) * T_e])
                            adb = eo.tile([128, NBW], F32, tag="adb")
                            nc.sync.dma_start(
                                out=adb[:],
                                in_=ad_stage[l][0:1, t * NB: t * NB + NBW]
                                .to_broadcast([128, NBW]))
                            ghs = ed.tile([128, T_e], F32, tag="ghs")
                            nc.gpsimd.ap_gather(
                                ghs[:], table[:], it[:, 0:T16],
                                channels=128, num_elems=NLP, d=1, num_idxs=T_e)
                            adg = eo.tile([128, T_e], F32, tag="adg")
                            nc.gpsimd.ap_gather(
                                adg[:], adb[:], it[:, T16:2 * T16],
                                channels=128, num_elems=NBW, d=1, num_idxs=T_e)
                            gb = ghs[:].bitcast(BF16)  # [128, 2*T_e]
                            tt = eo.tile([128, T_e], BF16, tag="tt")
                            nc.vector.tensor_add(tt[:], gb[:, 1::2], adg[:])
                            nc.vector.scalar_tensor_tensor(
                                tt[:], tt[:], NEG_SLOPE, tt[:],
                                op0=mybir.AluOpType.mult, op1=mybir.AluOpType.max)
                            pp = eo.tile([128, T_e], BF16, tag="pp")
                            nc.scalar.activation(pp[:], tt[:],
                                                 mybir.ActivationFunctionType.Exp)
                            ww = eo.tile([128, T_e], BF16, tag="ww")
                            nc.vector.tensor_mul(ww[:], pp[:], gb[:, 0::2])
                            sc = eo.tile([128, T_e, 2], F32, tag="sc")
                            nc.vector.tensor_tensor_scan(
                                sc[:, :, 0], mt[:], ww[:], 0.0,
                                mybir.AluOpType.mult, mybir.AluOpType.add)
                            nc.vector.tensor_tensor_scan(
                                sc[:, :, 1], mt[:], pp[:], 0.0,
                                mybir.AluOpType.mult, mybir.AluOpType.add)
                            bg = ed.tile([128, 512, 2], F32, tag="bg")
                            nc.gpsimd.ap_gather(
                                bg[:], sc[:], it[:, 2 * T16:],
                                channels=128, num_elems=T_e, d=2, num_idxs=512)
                            pu = edp.tile([16, 512], F32, tag="pu")
                            nc.tensor.matmul(pu[:], lhsT=selt[:], rhs=bg[:, :, 0],
                                             start=True, stop=True)
                            ps = edp.tile([16, 512], F32, tag="ps")
                            nc.tensor.matmul(ps[:], lhsT=selt[:], rhs=bg[:, :, 1],
                                             start=True, stop=True)
                            # add analytic self-loop terms, then normalize
                            wst = ed.tile([16, NB], F32, tag="wst")
                            nc.sync.dma_start(out=wst[:],
                                              in_=wself_stage[l][:, t * NB:(t + 1) * NB])
                            pst = ed.tile([16, NB], F32, tag="pst")
                            nc.sync.dma_start(
                                out=pst[:],
                                in_=pself_stage[l][0:1, t * NB:(t + 1) * NB]
                                .to_broadcast([16, NB]))
                            den = ed.tile([16, NB], F32, tag="den")
                            nc.vector.tensor_add(den[:], ps[:, :NB], pst[:])
                            rc = ed.tile([16, NB], F32, tag="rc")
                            nc.vector.reciprocal(rc[:], den[:])
                            num = ed.tile([16, NB], F32, tag="num")
                            nc.vector.tensor_add(num[:], pu[:, :NB], wst[:])
                            uv = ed.tile([16, NB], F32, tag="uv")
                            nc.vector.tensor_mul(uv[:], num[:], rc[:])
                            writer(t, uv)

            def write1(t, uv):
                nc.scalar.activation(out1T[:, t * NB:(t + 1) * NB], uv[:],
                                     mybir.ActivationFunctionType.Relu,
                                     bias=b1t[:, 0:1])

            def write2(t, uv):
                nc.vector.tensor_scalar_add(out2T[:, t * NB:(t + 1) * NB], uv[:],
                                            b2t[:, 0:1])

            run_layer(0, write1)

            # ---------------- phase A (layer 2): tables from out1T --------
            with (
                tc.tile_pool(name="pa2", bufs=2) as pa2,
                tc.tile_pool(name="pap2", bufs=2, space="PSUM") as pap2,
            ):
                for c in range(NCH):
                    sl = slice(c * CH, (c + 1) * CH)
                    ph = pap2.tile([16, CH], F32, tag="ph2")
                    nc.tensor.matmul(ph[:], lhsT=w2t[:], rhs=out1T[:, sl],
                                     start=True, stop=True)
                    h2ch = pa2.tile([16, CH], F32, tag="h2ch")
                    nc.vector.tensor_copy(h2ch[:], ph[:])
                    pas = pap2.tile([16, CH], F32, tag="pas2")
                    nc.tensor.matmul(pas[:], lhsT=a2t[:], rhs=h2ch[:], start=True, stop=True)
                    pad_ = pap2.tile([16, CH], F32, tag="pad2")
                    nc.tensor.matmul(pad_[:], lhsT=ad2t[:], rhs=h2ch[:], start=True, stop=True)
                    packed = pa2.tile([16, CH], F32, tag="packed2")
                    pb = packed[:].bitcast(BF16)
                    nc.vector.tensor_copy(pb[:, 0::2], h2ch[:])
                    nc.vector.tensor_copy(pb[:, 1::2], pas[:])
                    nc.sync.dma_start(out=ag_in[1][:, sl], in_=packed[:])
                    adfull = pa2.tile([16, CH], F32, tag="adfull2")
                    nc.vector.tensor_copy(adfull[:], pad_[:])
                    adrow = pa2.tile([1, CH], F32, tag="adrow2")
                    nc.vector.tensor_copy(adrow[:], adfull[0:1, :])
                    nc.sync.dma_start(out=ad_stage[1][0:1, sl], in_=adrow[:])
                    tself = pa2.tile([16, CH], F32, tag="tself2")
                    nc.vector.tensor_add(tself[:], pas[:], adfull[:])
                    nc.vector.scalar_tensor_tensor(
                        tself[:], tself[:], NEG_SLOPE, tself[:],
                        op0=mybir.AluOpType.mult, op1=mybir.AluOpType.max)
                    pselfc = pa2.tile([16, CH], F32, tag="pselfc2")
                    nc.scalar.activation(pselfc[:], tself[:],
                                         mybir.ActivationFunctionType.Exp)
                    wselfc = pa2.tile([16, CH], F32, tag="wselfc2")
                    nc.vector.tensor_mul(wselfc[:], pselfc[:], h2ch[:])
                    nc.sync.dma_start(out=wself_stage[1][:, sl], in_=wselfc[:])
                    nc.sync.dma_start(out=pself_stage[1][0:1, sl], in_=pselfc[0:1, :])
                nc.sync.dma_start(out=ad_stage[1][0:1, NL:], in_=zero16[:])

            if NL2 > NL:
                nc.vector.memset(out2T[:, NL:], 0.0)
            run_layer(1, write2)

            # ---------------- log_softmax + transpose + store -------------
            with (
                tc.tile_pool(name="fin", bufs=2) as fin,
                tc.tile_pool(name="finp", bufs=4, space="PSUM") as finp,
                tc.tile_pool(name="fino", bufs=1) as fino,
            ):
                nodemaj = fino.tile([128, NBLK, H], F32, tag="nodemaj")
                for j in range(NBLK):
                    ptp = finp.tile([128, 16], F32, tag="ptp")
                    nc.tensor.transpose(ptp[:], out2T[:, j * 128:(j + 1) * 128], idt[:])
                    nc.vector.tensor_copy(nodemaj[:, j, :], ptp[:, :H])
                mx = fin.tile([128, NBLK], F32, tag="mx")
                nc.vector.tensor_reduce(mx[:], nodemaj[:], axis=mybir.AxisListType.X,
                                        op=mybir.AluOpType.max)
                zz = fino.tile([128, NBLK, H], F32, tag="zz")
                nc.vector.tensor_sub(zz[:], nodemaj[:],
                                     mx[:, :, None].to_broadcast([128, NBLK, H]))
                es = fino.tile([128, NBLK, H], F32, tag="es")
                nc.scalar.activation(es[:], zz[:], mybir.ActivationFunctionType.Exp)
                sm = fin.tile([128, NBLK], F32, tag="sm")
                nc.vector.tensor_reduce(sm[:], es[:], axis=mybir.AxisListType.X,
                                        op=mybir.AluOpType.add)
                ls = fin.tile([128, NBLK], F32, tag="ls")
                nc.scalar.activation(ls[:], sm[:], mybir.ActivationFunctionType.Ln)
                outf = fino.tile([128, NBLK, H], F32, tag="outf")
                nc.vector.tensor_sub(outf[:], zz[:],
                                     ls[:, :, None].to_broadcast([128, NBLK, H]))
                nc.sync.dma_start(out=outp[:].rearrange("p (b h) -> p b h", h=H),
                                  in_=outf[:])

    nc.compile()
    if split:
        split_waits(nc, max_waits=max_waits, ctrl_max_waits=ctrl_max_waits)
    return nc


CTRL_TYPES = ("InstDrain", "InstNoOp", "InstHalt", "InstEventSemaphore")


def split_waits(nc, max_waits=2, ctrl_max_waits=1):
    """walrus in this container caps sync-waits per instruction; move excess
    waits onto preceding same-engine NoOps (each carrying one wait)."""
    for f in nc.m.functions:
        for bb in f.blocks:
            new_insts, changed = [], False
            for ins in bb.instructions:
                si = ins.sync_info
                cap = (ctrl_max_waits if type(ins).__name__ in CTRL_TYPES
                       else max_waits)
                if si is not None and si.on_wait is not None and len(si.on_wait) > cap:
                    waits = list(si.on_wait)
                    excess, keep = waits[:-cap] if cap else waits, waits[-cap:] if cap else []
                    for i, w in enumerate(excess):
                        nop = mybir.InstNoOp(name=f"{ins.name}-ws{i}", ins=[], outs=[])
                        nop.engine = ins.engine
                        nop.sync_info = mybir.SyncInfo(on_wait=[w], on_update=[])
                        new_insts.append(nop)
                    si.on_wait = keep
                    changed = True
                new_insts.append(ins)
            if changed:
                bb.instructions = new_insts
    # verify the rewrite stuck (pyo3 lists can copy-on-read)
    for f in nc.m.functions:
        for bb in f.blocks:
            for ins in bb.instructions:
                si = ins.sync_info
                cap = (ctrl_max_waits if type(ins).__name__ in CTRL_TYPES
                       else max_waits)
                assert si is None or si.on_wait is None or len(si.on_wait) <= cap, \
                    f"{ins.name}: {len(si.on_wait)} waits > {cap}"


# ------------------------------------------------------------ input packing

def make_in_maps(inputs, cfg, per_core):
    NC, NL, H, F = cfg["NCORES"], cfg["NL"], cfg["H"], cfg["F"]
    x = np.asarray(inputs["x"], dtype=np.float32)
    xt_full = np.ascontiguousarray(x.T)  # [F, N]
    sel = np.zeros((128, 16), dtype=np.float32)
    sel[np.arange(128), np.arange(128) % 16] = 1.0
    shared = {
        "w1": np.ascontiguousarray(np.asarray(inputs["W1"], np.float32)),
        "w2": np.ascontiguousarray(np.asarray(inputs["W2"], np.float32)),
        "a1rep": np.ascontiguousarray(np.repeat(np.asarray(inputs["a_src1"], np.float32)[:, None], 16, 1)),
        "ad1rep": np.ascontiguousarray(np.repeat(np.asarray(inputs["a_dst1"], np.float32)[:, None], 16, 1)),
        "a2rep": np.ascontiguousarray(np.repeat(np.asarray(inputs["a_src2"], np.float32)[:, None], 16, 1)),
        "ad2rep": np.ascontiguousarray(np.repeat(np.asarray(inputs["a_dst2"], np.float32)[:, None], 16, 1)),
        "b1p": np.ascontiguousarray(np.asarray(inputs["b1"], np.float32)[:, None]),
        "b2p": np.ascontiguousarray(np.asarray(inputs["b2"], np.float32)[:, None]),
        "selp": sel,
        "identp": np.eye(16, dtype=np.float32),
    }
    in_maps = []
    for c in range(NC):
        m = dict(shared)
        m["xt"] = np.ascontiguousarray(xt_full[:, c * NL:(c + 1) * NL])
        m["idxs"] = per_core[c]["idxs"]
        m["mask"] = per_core[c]["mask"]
        in_maps.append(m)
    return in_maps


def unshard_output(results, cfg):
    NC, NL, H = cfg["NCORES"], cfg["NL"], cfg["H"]
    NBLK = math.ceil(NL / 128)
    parts = []
    for c in range(NC):
        a = np.asarray(results[c]["out"]).reshape(128, NBLK, H)
        a = a.transpose(1, 0, 2).reshape(NBLK * 128, H)[:NL]
        parts.append(a)
    return np.concatenate(parts, axis=0)


# ------------------------------------------------------------------- driver

_CACHE = {}


def run_on_hw(inputs, cfg, trace=False, tmpdir=None):
    import os
    import shutil
    from concourse.bass_utils import run_bass_kernel_spmd
    if tmpdir is not None and os.path.isdir(tmpdir):
        shutil.rmtree(tmpdir, ignore_errors=True)
    if tmpdir is not None:
        os.makedirs(tmpdir, exist_ok=True)
    T_e, per_core = host_prep(inputs["edge_index"], cfg)
    key = (cfg["N"], T_e)
    if key not in _CACHE:
        _CACHE[key] = build_nc(cfg, T_e)
    nc = _CACHE[key]
    in_maps = make_in_maps(inputs, cfg, per_core)
    res = run_bass_kernel_spmd(nc, in_maps, list(range(cfg["NCORES"])),
                               trace=trace, tmpdir=tmpdir)
    out = unshard_output(res.results, cfg)
    return out, res


def kernel(**inputs):
    out, _ = run_on_hw(inputs, FULL_CFG)
    return out.astype(np.float32)



# revision 4
# speedup vs baseline: 1.0473x; 1.0473x over previous
"""GAT (2-layer, heads=1) on 8 Trainium2 NeuronCores.

Strategy (1D node partition, per the sharding hint):
  - Nodes are split into 8 chunks of NL; core c owns dst-chunk c.
  - Layer tables h^T/alpha_src are computed per-chunk (feature-major via
    PE matmuls on x^T in bf16), AllGathered, and kept in SBUF as a
    per-partition-group table: partition group g (16 partitions) holds
    (h|alpha_src) of chunk g, packed two bf16 per fp32 word.
  - Edges are bucketed on the host by (dst-core, src-chunk, dst-block) and
    sorted by dst. Each edge tile covers one block of NB dst nodes for all
    8 src-chunk groups at once; per-edge gathers run on GPSIMD (ap_gather),
    per-edge math split across DVE (adds/muls/scans) and ACT (prelu/exp),
    and segment-sums via masked tensor_tensor_scan into interleaved bf16
    (ww|pp) pairs + one boundary gather (d=2) per node's last edge.
    One bf16 matmul against a 0/1 selection matrix sums the 8 groups for
    numerator and denominator at once; the per-node self-loop terms are
    accumulated into the same PSUM via a second (identity) matmul.
  - The edge loop is software-pipelined one tile deep so the GPSIMD queue
    never blocks the next tile's gathers behind the scan-dependent
    boundary gather, and layer-2 table generation (phase A2) is
    interleaved chunk-by-chunk into the layer-1 edge loop.
  - Softmax max-subtraction is skipped: attention logits here are O(1), and
    alpha = exp(e)/sum(exp(e)) is shift-invariant.

Host preprocessing only reorders/buckets edge indices (structure) and casts
inputs to bf16; it never computes float data dependent on device results.
"""

import math
import numpy as np

import ml_dtypes

from concourse import bass, bacc, mybir
import concourse.tile as tile

F32 = mybir.dt.float32
BF16 = mybir.dt.bfloat16
I16 = mybir.dt.int16

NEG_SLOPE = 0.2
# fp32 -29952.0 == 0xC6EA0000: low bf16 lane (h) = 0, high bf16 lane
# (alpha_src) = -29952 -> exp(lrelu(...)) == 0, so sentinel edges vanish.
SENTINEL_PACKED = -29952.0

FULL_CFG = dict(
    NCORES=8, N=100000, F=512, H=16,
    NL=12500, NB=500, NT=25, CH=500, NCH=25,
)


# ---------------------------------------------------------------- host prep

def _round_up(x, m):
    return (x + m - 1) // m * m


def host_prep(edge_index, cfg):
    """Bucket and sort edges; build device index/mask streams.

    Returns (T_e, per_core) where per_core[c] has 'idxs' [128, NT*IW] i16 and
    'mask' [128, NT*T_e] bf16.
    """
    NC, N, NL = cfg["NCORES"], cfg["N"], cfg["NL"]
    NB, NT = cfg["NB"], cfg["NT"]
    G = 8

    # Self-loops are NOT added here: their contribution is computed
    # analytically at the node level on-device (no gather needed).
    src = np.asarray(edge_index[0], dtype=np.int64)
    dst = np.asarray(edge_index[1], dtype=np.int64)

    core = dst // NL
    grp = src // NL
    blk = (dst % NL) // NB
    order = np.lexsort((src, dst, grp, core))
    src, dst, core, grp, blk = (a[order] for a in (src, dst, core, grp, blk))

    bucket = (core * G + grp) * NT + blk
    nbuck = NC * G * NT
    counts = np.bincount(bucket, minlength=nbuck)
    T_e = _round_up(int(counts.max()) + 16, 128)
    assert T_e <= 32767

    starts = np.zeros(nbuck, dtype=np.int64)
    starts[1:] = np.cumsum(counts)[:-1]
    pos = np.arange(src.size) - starts[bucket]

    prev_same = np.zeros(src.size, dtype=bool)
    prev_same[1:] = (bucket[1:] == bucket[:-1]) & (dst[1:] == dst[:-1])
    is_last = np.ones(src.size, dtype=bool)
    is_last[:-1] = ~((bucket[1:] == bucket[:-1]) & (dst[1:] == dst[:-1]))

    srcl = np.full((NC, G, NT, T_e), NL, dtype=np.int16)
    dstl = np.full((NC, G, NT, T_e), NB, dtype=np.int16)
    maskv = np.zeros((NC, G, NT, T_e), dtype=np.float32)
    bnd = np.full((NC, G, NT, 512), T_e - 1, dtype=np.int16)

    c_, g_, b_ = core, grp, blk
    srcl[c_, g_, b_, pos] = (src % NL).astype(np.int16)
    dstl[c_, g_, b_, pos] = ((dst % NL) % NB).astype(np.int16)
    maskv[c_[prev_same], g_[prev_same], b_[prev_same], pos[prev_same]] = 1.0
    bnd[c_[is_last], g_[is_last], b_[is_last],
        ((dst % NL) % NB)[is_last]] = pos[is_last].astype(np.int16)

    def wrap(a, w):
        # [NC, G, NT, w] -> [NC, 128, NT, w//16]; w16[c, 16g+p, t, s] = a[c, g, t, s*16+p]
        n = a.shape[-1]
        return (a.reshape(NC, G, NT, n // 16, 16)
                 .transpose(0, 1, 4, 2, 3)
                 .reshape(NC, 128, NT, n // 16))

    src_w, dst_w, bnd_w = wrap(srcl, T_e), wrap(dstl, T_e), wrap(bnd, 512)
    idxs = np.concatenate([src_w, dst_w, bnd_w], axis=3)  # [NC,128,NT,IW]
    IW = idxs.shape[3]
    idxs = np.ascontiguousarray(idxs.reshape(NC, 128, NT * IW))

    mask_rep = np.repeat(maskv, 16, axis=1)  # [NC, 128, NT, T_e]
    mask_rep = np.ascontiguousarray(
        mask_rep.reshape(NC, 128, NT * T_e).astype(ml_dtypes.bfloat16))

    per_core = [{"idxs": idxs[c], "mask": mask_rep[c]} for c in range(NC)]
    return T_e, per_core


# ------------------------------------------------------------- device build

def build_nc(cfg, T_e, max_waits=2, ctrl_max_waits=1, split=True):
    NC, N, F, H = cfg["NCORES"], cfg["N"], cfg["F"], cfg["H"]
    NL, NB, NT = cfg["NL"], cfg["NB"], cfg["NT"]
    CH, NCH = cfg["CH"], cfg["NCH"]
    KB = F // 128
    NLP = NL + 16               # table width incl. sentinel
    NBW = NB + 16               # per-tile alpha_dst table width
    T16 = T_e // 16
    IW = 2 * T16 + 512 // 16
    NBLK = math.ceil(NL / 128)
    NL2 = NBLK * 128
    rg = [list(range(NC))]
    Prelu = mybir.ActivationFunctionType.Prelu
    Exp = mybir.ActivationFunctionType.Exp
    Copy = mybir.ActivationFunctionType.Copy
    Relu = mybir.ActivationFunctionType.Relu
    Ident = mybir.ActivationFunctionType.Identity

    nc = bacc.Bacc("TRN2", target_bir_lowering=False)

    xt = nc.declare_dram_parameter("xt", [F, NL], BF16, isOutput=False)
    w1 = nc.declare_dram_parameter("w1", [F, H], BF16, isOutput=False)
    w2 = nc.declare_dram_parameter("w2", [H, H], BF16, isOutput=False)
    a1rep = nc.declare_dram_parameter("a1rep", [H, 16], BF16, isOutput=False)
    ad1rep = nc.declare_dram_parameter("ad1rep", [H, 16], BF16, isOutput=False)
    a2rep = nc.declare_dram_parameter("a2rep", [H, 16], BF16, isOutput=False)
    ad2rep = nc.declare_dram_parameter("ad2rep", [H, 16], BF16, isOutput=False)
    b1p = nc.declare_dram_parameter("b1p", [H, 1], F32, isOutput=False)
    b2p = nc.declare_dram_parameter("b2p", [H, 1], F32, isOutput=False)
    selp = nc.declare_dram_parameter("selp", [128, 16], BF16, isOutput=False)
    identp = nc.declare_dram_parameter("identp", [16, 16], BF16, isOutput=False)
    idxsp = nc.declare_dram_parameter("idxs", [128, NT * IW], I16, isOutput=False)
    maskp = nc.declare_dram_parameter("mask", [128, NT * T_e], BF16, isOutput=False)
    outp = nc.declare_dram_parameter("out", [128, NBLK * H], F32, isOutput=True)

    ag_in = [nc.dram_tensor(f"ag_in{l}", [16, NL], F32) for l in (1, 2)]
    ag_out = [nc.dram_tensor(f"ag_out{l}", [128, NL], F32, addr_space="Shared")
              for l in (1, 2)]
    ad_stage = [nc.dram_tensor(f"ad_stage{l}", [1, NL + NBW], F32) for l in (1, 2)]
    # interleaved (wself | pself) bf16 pairs per node, 16 feature rows
    wp_stage = [nc.dram_tensor(f"wp_stage{l}", [16, 2 * (NL + 16)], BF16)
                for l in (1, 2)]

    with tile.TileContext(nc, num_cores=NC) as tc:
        with tc.tile_pool(name="const", bufs=1) as cpool:
            w1t = cpool.tile([128, KB, H], BF16)
            nc.sync.dma_start(out=w1t[:], in_=w1[:].rearrange("(b p) h -> p b h", p=128))
            w2t = cpool.tile([16, H], BF16)
            nc.sync.dma_start(out=w2t[:], in_=w2[:])
            a1t = cpool.tile([16, 16], BF16)
            nc.sync.dma_start(out=a1t[:], in_=a1rep[:])
            ad1t = cpool.tile([16, 16], BF16)
            nc.sync.dma_start(out=ad1t[:], in_=ad1rep[:])
            a2t = cpool.tile([16, 16], BF16)
            nc.sync.dma_start(out=a2t[:], in_=a2rep[:])
            ad2t = cpool.tile([16, 16], BF16)
            nc.sync.dma_start(out=ad2t[:], in_=ad2rep[:])
            b1t = cpool.tile([16, 1], F32)
            nc.sync.dma_start(out=b1t[:], in_=b1p[:])
            b2t = cpool.tile([16, 1], F32)
            nc.sync.dma_start(out=b2t[:], in_=b2p[:])
            selt = cpool.tile([128, 16], BF16)
            nc.sync.dma_start(out=selt[:], in_=selp[:])
            idt = cpool.tile([16, 16], BF16)
            nc.sync.dma_start(out=idt[:], in_=identp[:])
            zero16 = cpool.tile([1, NBW], F32)
            nc.vector.memset(zero16[:], 0.0)

            # out1T and out2T share one slot: all out1T reads (phase A2)
            # complete before any out2T write (layer-2 edge phase).
            out1T = cpool.tile([16, NL2], BF16, tag="outT")
            out2T = cpool.tile([16, NL2], BF16, tag="outT")

            def phase_a_chunk(l, c, pa, pap, h_matmul):
                """Emit table-gen ops for chunk c of layer l."""
                sl = slice(c * CH, (c + 1) * CH)
                ph = pap.tile([16, CH], F32, tag="ph")
                h_matmul(ph, sl)
                hch = pa.tile([16, CH], BF16, tag="hch")
                nc.scalar.activation(hch[:], ph[:], Copy)
                at, adt = (a1t, ad1t) if l == 0 else (a2t, ad2t)
                pas = pap.tile([16, CH], F32, tag="pas")
                nc.tensor.matmul(pas[:], lhsT=at[:], rhs=hch[:], start=True, stop=True)
                pad_ = pap.tile([16, CH], F32, tag="pad")
                nc.tensor.matmul(pad_[:], lhsT=adt[:], rhs=hch[:], start=True, stop=True)
                packed = pa.tile([16, CH], F32, tag="packed")
                pb = packed[:].bitcast(BF16)
                nc.scalar.activation(pb[:, 0::2], hch[:], Copy)
                nc.scalar.activation(pb[:, 1::2], pas[:], Copy)
                nc.sync.dma_start(out=ag_in[l][:, sl], in_=packed[:])
                adfull = pa.tile([16, CH], F32, tag="adfull")
                nc.scalar.activation(adfull[:], pad_[:], Copy)
                nc.sync.dma_start(out=ad_stage[l][0:1, sl], in_=adfull[0:1, :])
                # analytic self-loop contribution for this chunk
                tself = pa.tile([16, CH], F32, tag="tself")
                nc.vector.tensor_add(tself[:], pas[:], adfull[:])
                nc.scalar.activation(tself[:], tself[:], Prelu, alpha=NEG_SLOPE)
                nc.scalar.activation(tself[:], tself[:], Exp)
                wsp = pa.tile([16, 2 * CH], BF16, tag="wsp")
                nc.vector.tensor_mul(wsp[:, 0::2], tself[:], hch[:])
                nc.scalar.activation(wsp[:, 1::2], tself[:], Copy)
                nc.sync.dma_start(
                    out=wp_stage[l][:, 2 * c * CH:2 * (c + 1) * CH], in_=wsp[:])

            # ---------------- phase A (layer 1): tables from x^T ----------
            with (
                tc.tile_pool(name="pa", bufs=2) as pa,
                tc.tile_pool(name="pap", bufs=2, space="PSUM") as pap,
            ):
                def h1_matmul(ph, sl):
                    xt_t = pa.tile([128, KB, CH], BF16, tag="xt")
                    nc.sync.dma_start(
                        out=xt_t[:],
                        in_=xt[:, sl].rearrange("(b p) n -> p b n", p=128))
                    for b in range(KB):
                        nc.tensor.matmul(ph[:], lhsT=w1t[:, b, :], rhs=xt_t[:, b, :],
                                         start=(b == 0), stop=(b == KB - 1))
                for c in range(NCH):
                    phase_a_chunk(0, c, pa, pap, h1_matmul)
                nc.sync.dma_start(out=ad_stage[0][0:1, NL:], in_=zero16[:])

            def run_layer(l, writer, post_tile=None):
                nc.gpsimd.collective_compute(
                    "AllGather", mybir.AluOpType.bypass, replica_groups=rg,
                    ins=[ag_in[l][:]], outs=[ag_out[l][:]])
                with tc.tile_pool(name=f"tab{l}", bufs=1) as tabp:
                    table = tabp.tile([128, NLP], F32, tag="table")
                    nc.sync.dma_start(out=table[:, :NL], in_=ag_out[l][:])
                    nc.vector.memset(table[:, NL:], SENTINEL_PACKED)
                    with (
                        tc.tile_pool(name=f"st{l}", bufs=2) as st,
                        tc.tile_pool(name=f"ed{l}", bufs=2) as ed,
                        tc.tile_pool(name=f"sq{l}", bufs=2) as sq,
                        tc.tile_pool(name=f"edp{l}", bufs=2, space="PSUM") as edp,
                    ):
                        prev = None
                        for t in range(NT + 1):
                            cur = None
                            if t < NT:
                                # stream in + gathers for tile t
                                it = st.tile([128, IW], I16, tag="it")
                                nc.sync.dma_start(out=it[:], in_=idxsp[:, t * IW:(t + 1) * IW])
                                mt = st.tile([128, T_e], BF16, tag="mt")
                                nc.sync.dma_start(out=mt[:], in_=maskp[:, t * T_e:(t + 1) * T_e])
                                adb = st.tile([128, NBW], F32, tag="adb")
                                nc.sync.dma_start(
                                    out=adb[:],
                                    in_=ad_stage[l][0:1, t * NB: t * NB + NBW]
                                    .to_broadcast([128, NBW]))
                                wpst = st.tile([16, 1024], BF16, tag="wpst")
                                nc.sync.dma_start(
                                    out=wpst[:],
                                    in_=wp_stage[l][:, 2 * t * NB: 2 * t * NB + 1024])
                                ghs = ed.tile([128, T_e], F32, tag="ghs")
                                nc.gpsimd.ap_gather(
                                    ghs[:], table[:], it[:, 0:T16],
                                    channels=128, num_elems=NLP, d=1, num_idxs=T_e)
                                adg = ed.tile([128, T_e], F32, tag="adg")
                                nc.gpsimd.ap_gather(
                                    adg[:], adb[:], it[:, T16:2 * T16],
                                    channels=128, num_elems=NBW, d=1, num_idxs=T_e)
                                cur = dict(it=it, mt=mt, wpst=wpst, ghs=ghs, adg=adg, t=t)

                            if prev is not None:
                                # tail of tile t-1: boundary gather, matmuls,
                                # normalize, write (+ interleaved phase-A2 work)
                                pt = prev["t"]
                                bg = ed.tile([128, 1024], BF16, tag="bg")
                                nc.gpsimd.ap_gather(
                                    bg[:].rearrange("p (n d) -> p n d", d=2),
                                    prev["sc"][:], prev["it"][:, 2 * T16:],
                                    channels=128, num_elems=T_e, d=2, num_idxs=512)
                                pq = edp.tile([16, 1024], F32, tag="pq")
                                for h0 in (0, 512):
                                    hs = slice(h0, h0 + 512)
                                    nc.tensor.matmul(pq[:, hs], lhsT=selt[:],
                                                     rhs=bg[:, hs],
                                                     start=True, stop=False)
                                    nc.tensor.matmul(pq[:, hs], lhsT=idt[:],
                                                     rhs=prev["wpst"][:, hs],
                                                     start=False, stop=True)
                                prev.update(bg=bg, pq=pq)

                            if t < NT:
                                # per-edge math for tile t
                                gb = cur["ghs"][:].bitcast(BF16)  # [128, 2*T_e]
                                z = ed.tile([128, T_e], BF16, tag="z")
                                nc.vector.tensor_add(z[:], gb[:, 1::2], cur["adg"][:])
                                nc.scalar.activation(z[:], z[:], Prelu, alpha=NEG_SLOPE)
                                pp = ed.tile([128, T_e], BF16, tag="pp")
                                nc.scalar.activation(pp[:], z[:], Exp)

                            if prev is not None:
                                # normalize + write t-1 (fills the DVE gap while
                                # ACT computes prelu/exp for tile t)
                                pt = prev["t"]
                                pq = prev["pq"]
                                pqv = pq[:].rearrange("p (n d) -> p n d", d=2)
                                rc = sq.tile([16, 512], F32, tag="rc")
                                nc.vector.reciprocal_approx_fast(out=rc[:], in_=pqv[:, :, 1])
                                uv = sq.tile([16, NB], F32, tag="uv")
                                nc.vector.tensor_mul(uv[:], pqv[:, :NB, 0], rc[:, :NB])
                                writer(pt, uv)
                                if post_tile is not None:
                                    post_tile(pt)

                            if t < NT:
                                ww = ed.tile([128, T_e], BF16, tag="ww")
                                nc.vector.tensor_mul(ww[:], pp[:], gb[:, 0::2])
                                sc = ed.tile([128, T_e, 2], BF16, tag="sc")
                                nc.vector.tensor_tensor_scan(
                                    sc[:, :, 0], cur["mt"][:], ww[:], 0.0,
                                    mybir.AluOpType.mult, mybir.AluOpType.add)
                                nc.vector.tensor_tensor_scan(
                                    sc[:, :, 1], cur["mt"][:], pp[:], 0.0,
                                    mybir.AluOpType.mult, mybir.AluOpType.add)
                                cur["sc"] = sc
                            prev = cur

            def write1(t, uv):
                nc.scalar.activation(out1T[:, t * NB:(t + 1) * NB], uv[:],
                                     Relu, bias=b1t[:, 0:1])

            def write2(t, uv):
                nc.scalar.activation(out2T[:, t * NB:(t + 1) * NB], uv[:],
                                     Ident, bias=b2t[:, 0:1])

            # phase A2 (layer-2 tables) interleaved into the layer-1 loop:
            # chunk c reads out1T columns written by edge tile c.
            with (
                tc.tile_pool(name="pa2", bufs=2) as pa2,
                tc.tile_pool(name="pap2", bufs=1, space="PSUM") as pap2,
            ):
                def a2_chunk(c):
                    def h2_matmul(ph, sl):
                        nc.tensor.matmul(ph[:], lhsT=w2t[:], rhs=out1T[:, sl],
                                         start=True, stop=True)
                    phase_a_chunk(1, c, pa2, pap2, h2_matmul)
                    if c == NCH - 1:
                        nc.sync.dma_start(out=ad_stage[1][0:1, NL:], in_=zero16[:])

                run_layer(0, write1, post_tile=a2_chunk)

            if NL2 > NL:
                nc.vector.memset(out2T[:, NL:], 0.0)
            run_layer(1, write2)

            # ---------------- log_softmax + transpose + store -------------
            with (
                tc.tile_pool(name="fin", bufs=2) as fin,
                tc.tile_pool(name="finp", bufs=4, space="PSUM") as finp,
                tc.tile_pool(name="fino", bufs=1) as fino,
            ):
                nodemaj = fino.tile([128, NBLK, H], F32, tag="nodemaj")
                for j in range(NBLK):
                    ptp = finp.tile([128, 16], BF16, tag="ptp")
                    nc.tensor.transpose(ptp[:], out2T[:, j * 128:(j + 1) * 128], idt[:])
                    nc.vector.tensor_copy(nodemaj[:, j, :], ptp[:, :H])
                mx = fin.tile([128, NBLK], F32, tag="mx")
                nc.vector.tensor_reduce(mx[:], nodemaj[:], axis=mybir.AxisListType.X,
                                        op=mybir.AluOpType.max)
                zz = fino.tile([128, NBLK, H], F32, tag="zz")
                nc.vector.tensor_sub(zz[:], nodemaj[:],
                                     mx[:, :, None].to_broadcast([128, NBLK, H]))
                es = fino.tile([128, NBLK, H], F32, tag="es")
                nc.scalar.activation(es[:], zz[:], Exp)
                sm = fin.tile([128, NBLK], F32, tag="sm")
                nc.vector.tensor_reduce(sm[:], es[:], axis=mybir.AxisListType.X,
                                        op=mybir.AluOpType.add)
                ls = fin.tile([128, NBLK], F32, tag="ls")
                nc.scalar.activation(ls[:], sm[:], mybir.ActivationFunctionType.Ln)
                outf = fino.tile([128, NBLK, H], F32, tag="outf")
                nc.vector.tensor_sub(outf[:], zz[:],
                                     ls[:, :, None].to_broadcast([128, NBLK, H]))
                nc.sync.dma_start(out=outp[:].rearrange("p (b h) -> p b h", h=H),
                                  in_=outf[:])

    nc.compile()
    if split:
        split_waits(nc, max_waits=max_waits, ctrl_max_waits=ctrl_max_waits)
    return nc


CTRL_TYPES = ("InstDrain", "InstNoOp", "InstHalt", "InstEventSemaphore")


def split_waits(nc, max_waits=2, ctrl_max_waits=1):
    """walrus in this container caps sync-waits per instruction; move excess
    waits onto preceding same-engine NoOps (each carrying one wait)."""
    for f in nc.m.functions:
        for bb in f.blocks:
            new_insts, changed = [], False
            for ins in bb.instructions:
                si = ins.sync_info
                cap = (ctrl_max_waits if type(ins).__name__ in CTRL_TYPES
                       else max_waits)
                if si is not None and si.on_wait is not None and len(si.on_wait) > cap:
                    waits = list(si.on_wait)
                    excess, keep = waits[:-cap] if cap else waits, waits[-cap:] if cap else []
                    for i, w in enumerate(excess):
                        nop = mybir.InstNoOp(name=f"{ins.name}-ws{i}", ins=[], outs=[])
                        nop.engine = ins.engine
                        nop.sync_info = mybir.SyncInfo(on_wait=[w], on_update=[])
                        new_insts.append(nop)
                    si.on_wait = keep
                    changed = True
                new_insts.append(ins)
            if changed:
                bb.instructions = new_insts
    # verify the rewrite stuck (pyo3 lists can copy-on-read)
    for f in nc.m.functions:
        for bb in f.blocks:
            for ins in bb.instructions:
                si = ins.sync_info
                cap = (ctrl_max_waits if type(ins).__name__ in CTRL_TYPES
                       else max_waits)
                assert si is None or si.on_wait is None or len(si.on_wait) <= cap, \
                    f"{ins.name}: {len(si.on_wait)} waits > {cap}"


# ------------------------------------------------------------ input packing

def make_in_maps(inputs, cfg, per_core):
    NC, NL, H, F = cfg["NCORES"], cfg["NL"], cfg["H"], cfg["F"]
    BF = ml_dtypes.bfloat16
    x = np.asarray(inputs["x"], dtype=np.float32)
    xt_full = np.ascontiguousarray(x.T.astype(BF))  # [F, N] bf16
    sel = np.zeros((128, 16), dtype=BF)
    sel[np.arange(128), np.arange(128) % 16] = 1.0

    def rep16(v):
        return np.ascontiguousarray(
            np.repeat(np.asarray(v, np.float32)[:, None], 16, 1).astype(BF))

    shared = {
        "w1": np.ascontiguousarray(np.asarray(inputs["W1"], np.float32).astype(BF)),
        "w2": np.ascontiguousarray(np.asarray(inputs["W2"], np.float32).astype(BF)),
        "a1rep": rep16(inputs["a_src1"]),
        "ad1rep": rep16(inputs["a_dst1"]),
        "a2rep": rep16(inputs["a_src2"]),
        "ad2rep": rep16(inputs["a_dst2"]),
        "b1p": np.ascontiguousarray(np.asarray(inputs["b1"], np.float32)[:, None]),
        "b2p": np.ascontiguousarray(np.asarray(inputs["b2"], np.float32)[:, None]),
        "selp": sel,
        "identp": np.eye(16, dtype=BF),
    }
    in_maps = []
    for c in range(NC):
        m = dict(shared)
        m["xt"] = np.ascontiguousarray(xt_full[:, c * NL:(c + 1) * NL])
        m["idxs"] = per_core[c]["idxs"]
        m["mask"] = per_core[c]["mask"]
        in_maps.append(m)
    return in_maps


def unshard_output(results, cfg):
    NC, NL, H = cfg["NCORES"], cfg["NL"], cfg["H"]
    NBLK = math.ceil(NL / 128)
    parts = []
    for c in range(NC):
        a = np.asarray(results[c]["out"]).reshape(128, NBLK, H)
        a = a.transpose(1, 0, 2).reshape(NBLK * 128, H)[:NL]
        parts.append(a)
    return np.concatenate(parts, axis=0)


# ------------------------------------------------------------------- driver

_CACHE = {}


def run_on_hw(inputs, cfg, trace=False, tmpdir=None):
    import os
    import shutil
    from concourse.bass_utils import run_bass_kernel_spmd
    if tmpdir is not None and os.path.isdir(tmpdir):
        shutil.rmtree(tmpdir, ignore_errors=True)
    if tmpdir is not None:
        os.makedirs(tmpdir, exist_ok=True)
    T_e, per_core = host_prep(inputs["edge_index"], cfg)
    key = (cfg["N"], T_e)
    if key not in _CACHE:
        _CACHE[key] = build_nc(cfg, T_e)
    nc = _CACHE[key]
    in_maps = make_in_maps(inputs, cfg, per_core)
    res = run_bass_kernel_spmd(nc, in_maps, list(range(cfg["NCORES"])),
                               trace=trace, tmpdir=tmpdir)
    out = unshard_output(res.results, cfg)
    return out, res


def kernel(**inputs):
    out, _ = run_on_hw(inputs, FULL_CFG)
    return out.astype(np.float32)


# revision 17
# speedup vs baseline: 1.6053x; 1.5328x over previous
"""GAT (2-layer, heads=1) on 8 Trainium2 NeuronCores.

Strategy (1D node partition, per the sharding hint):
  - Nodes are split into 8 chunks of NL; core c owns dst-chunk c.
  - Layer tables h^T/alpha_src are computed per-chunk (feature-major via
    PE matmuls on x^T in bf16), AllGathered, and kept in SBUF as a
    per-partition-group table: partition group g (16 partitions) holds
    (h|alpha_src) of chunk g, packed two bf16 per fp32 word.
  - Edges are bucketed on the host by (dst-core, src-chunk, dst-block) and
    sorted by dst. Each edge tile covers one block of NB dst nodes for all
    8 src-chunk groups at once; per-edge gathers run on GPSIMD (ap_gather),
    per-edge math split across DVE (adds/muls/scans) and ACT (prelu/exp),
    and segment-sums via masked tensor_tensor_scan into interleaved bf16
    (ww|pp) pairs + one boundary gather (d=2) per node's last edge.
    One bf16 matmul against a 0/1 selection matrix sums the 8 groups for
    numerator and denominator at once; the per-node self-loop terms are
    accumulated into the same PSUM via a second (identity) matmul.
  - The edge loop is software-pipelined one tile deep so the GPSIMD queue
    never blocks the next tile's gathers behind the scan-dependent
    boundary gather, and layer-2 table generation (phase A2) is
    interleaved chunk-by-chunk into the layer-1 edge loop.
  - Softmax max-subtraction is skipped: attention logits here are O(1), and
    alpha = exp(e)/sum(exp(e)) is shift-invariant.

Host preprocessing only reorders/buckets edge indices (structure) and casts
inputs to bf16; it never computes float data dependent on device results.
"""

import math
import numpy as np

import ml_dtypes

from concourse import bass, bacc, mybir
import concourse.tile as tile

F32 = mybir.dt.float32
BF16 = mybir.dt.bfloat16
I16 = mybir.dt.int16

NEG_SLOPE = 0.2
# fp32 -29952.0 == 0xC6EA0000: low bf16 lane (h) = 0, high bf16 lane
# (alpha_src) = -29952 -> exp(lrelu(...)) == 0, so sentinel edges vanish.
SENTINEL_PACKED = -29952.0

FULL_CFG = dict(
    NCORES=8, N=100000, F=512, H=16,
    NL=12500, NB=500, NT=25, CH=500, NCH=25,
)


# ---------------------------------------------------------------- host prep

def _round_up(x, m):
    return (x + m - 1) // m * m


def balance_permutation(edge_index, cfg):
    """Per-core node relabeling: deal dsts round-robin (by degree rank) across
    the NT blocks so (group, block) bucket sizes even out. Returns perm[N]:
    perm[old_global] = new_global (new id stays within the old node's core)."""
    NC, N, NL = cfg["NCORES"], cfg["N"], cfg["NL"]
    NB, NT = cfg["NB"], cfg["NT"]
    src = np.asarray(edge_index[0], dtype=np.int64)
    dst = np.asarray(edge_index[1], dtype=np.int64)
    deg = np.bincount(dst, minlength=N)
    perm = np.empty(N, dtype=np.int64)
    for c in range(NC):
        lo = c * NL
        d = deg[lo:lo + NL]
        order = np.argsort(-d, kind="stable")   # local ids by degree desc
        rank = np.empty(NL, dtype=np.int64)
        rank[order] = np.arange(NL)
        blk = rank % NT
        slot = rank // NT
        perm[lo:lo + NL] = lo + blk * NB + slot
    return perm


def host_prep(edge_index, cfg, perm):
    """Bucket and sort (relabeled) edges; build device index/mask/scatter
    streams.

    Returns (T_e, per_core) where per_core[c] has:
      'idxs'  [128, NT*IW] i16   (src-local wrapped + boundary wrapped)
      'mask'  [128, NT*T_e] bf16
      'sidxa','sidxb' [128, NT*512] i16  (run-start scatter indices, halves)
    """
    NC, N, NL = cfg["NCORES"], cfg["N"], cfg["NL"]
    NB, NT = cfg["NB"], cfg["NT"]
    G = 8

    # Self-loops are NOT added here: their contribution is computed
    # analytically at the node level on-device (no gather needed).
    src = perm[np.asarray(edge_index[0], dtype=np.int64)]
    dst = perm[np.asarray(edge_index[1], dtype=np.int64)]

    core = dst // NL
    grp = src // NL
    blk = (dst % NL) // NB
    order = np.lexsort((src, dst, grp, core))
    src, dst, core, grp, blk = (a[order] for a in (src, dst, core, grp, blk))

    bucket = (core * G + grp) * NT + blk
    nbuck = NC * G * NT
    counts = np.bincount(bucket, minlength=nbuck)
    T_e = _round_up(int(counts.max()) + 16, 128)
    assert T_e <= 32767

    starts = np.zeros(nbuck, dtype=np.int64)
    starts[1:] = np.cumsum(counts)[:-1]
    pos = np.arange(src.size) - starts[bucket]

    prev_same = np.zeros(src.size, dtype=bool)
    prev_same[1:] = (bucket[1:] == bucket[:-1]) & (dst[1:] == dst[:-1])
    is_first = ~prev_same
    is_last = np.ones(src.size, dtype=bool)
    is_last[:-1] = ~((bucket[1:] == bucket[:-1]) & (dst[1:] == dst[:-1]))

    srcl = np.full((NC, G, NT, T_e), NL, dtype=np.int16)
    maskv = np.zeros((NC, G, NT, T_e), dtype=np.float32)
    bnd = np.full((NC, G, NT, 512), T_e - 1, dtype=np.int16)
    start = np.full((NC, G, NT, 512), -1, dtype=np.int64)

    c_, g_, b_ = core, grp, blk
    dslot = ((dst % NL) % NB)
    srcl[c_, g_, b_, pos] = (src % NL).astype(np.int16)
    maskv[c_[prev_same], g_[prev_same], b_[prev_same], pos[prev_same]] = 1.0
    bnd[c_[is_last], g_[is_last], b_[is_last],
        dslot[is_last]] = pos[is_last].astype(np.int16)
    start[c_[is_first], g_[is_first], b_[is_first],
          dslot[is_first]] = pos[is_first]

    half = T_e // 2
    sidxa = np.where((start >= 0) & (start < half), start, -1).astype(np.int16)
    sidxb = np.where(start >= half, start - half, -1).astype(np.int16)

    def wrap(a, w):
        # [NC, G, NT, w] -> [NC, 128, NT, w//16]; w16[c, 16g+p, t, s] = a[c, g, t, s*16+p]
        n = a.shape[-1]
        return (a.reshape(NC, G, NT, n // 16, 16)
                 .transpose(0, 1, 4, 2, 3)
                 .reshape(NC, 128, NT, n // 16))

    src_w, bnd_w = wrap(srcl, T_e), wrap(bnd, 512)
    idxs = np.concatenate([src_w, bnd_w], axis=3)  # [NC,128,NT,IW]
    IW = idxs.shape[3]
    idxs = np.ascontiguousarray(idxs.reshape(NC, 128, NT * IW))

    mask_rep = np.repeat(maskv, 16, axis=1)  # [NC, 128, NT, T_e]
    mask_rep = np.ascontiguousarray(
        mask_rep.reshape(NC, 128, NT * T_e).astype(ml_dtypes.bfloat16))

    # scatter indices are per-partition (not wrapped): replicate per group
    sidxa = np.ascontiguousarray(
        np.repeat(sidxa, 16, axis=1).reshape(NC, 128, NT * 512))
    sidxb = np.ascontiguousarray(
        np.repeat(sidxb, 16, axis=1).reshape(NC, 128, NT * 512))

    per_core = [{"idxs": idxs[c], "mask": mask_rep[c],
                 "sidxa": sidxa[c], "sidxb": sidxb[c]} for c in range(NC)]
    return T_e, per_core


# ------------------------------------------------------------- device build

def build_nc(cfg, T_e, max_waits=2, ctrl_max_waits=1, split=True):
    NC, N, F, H = cfg["NCORES"], cfg["N"], cfg["F"], cfg["H"]
    NL, NB, NT = cfg["NL"], cfg["NB"], cfg["NT"]
    CH, NCH = cfg["CH"], cfg["NCH"]
    KB = F // 128
    NLP = NL + 16               # table width incl. sentinel
    NBW = 512                   # per-tile alpha_dst table width
    T16 = T_e // 16
    IW = T16 + 512 // 16
    NBLK = math.ceil(NL / 128)
    NL2 = NBLK * 128
    rg = [list(range(NC))]
    Prelu = mybir.ActivationFunctionType.Prelu
    Exp = mybir.ActivationFunctionType.Exp
    Copy = mybir.ActivationFunctionType.Copy
    Relu = mybir.ActivationFunctionType.Relu
    Ident = mybir.ActivationFunctionType.Identity

    nc = bacc.Bacc("TRN2", target_bir_lowering=False)

    xt = nc.declare_dram_parameter("xt", [F, NL], BF16, isOutput=False)
    w1 = nc.declare_dram_parameter("w1", [F, H], BF16, isOutput=False)
    w2 = nc.declare_dram_parameter("w2", [H, H], BF16, isOutput=False)
    a1rep = nc.declare_dram_parameter("a1rep", [H, 16], BF16, isOutput=False)
    ad1rep = nc.declare_dram_parameter("ad1rep", [H, 16], BF16, isOutput=False)
    a2rep = nc.declare_dram_parameter("a2rep", [H, 16], BF16, isOutput=False)
    ad2rep = nc.declare_dram_parameter("ad2rep", [H, 16], BF16, isOutput=False)
    b1p = nc.declare_dram_parameter("b1p", [H, 1], F32, isOutput=False)
    b2p = nc.declare_dram_parameter("b2p", [H, 1], F32, isOutput=False)
    selp = nc.declare_dram_parameter("selp", [128, 16], BF16, isOutput=False)
    identp = nc.declare_dram_parameter("identp", [16, 16], BF16, isOutput=False)
    idxsp = nc.declare_dram_parameter("idxs", [128, NT * IW], I16, isOutput=False)
    maskp = nc.declare_dram_parameter("mask", [128, NT * T_e], BF16, isOutput=False)
    sidxap = nc.declare_dram_parameter("sidxa", [128, NT * 512], I16, isOutput=False)
    sidxbp = nc.declare_dram_parameter("sidxb", [128, NT * 512], I16, isOutput=False)
    outp = nc.declare_dram_parameter("out", [128, NBLK * H], F32, isOutput=True)

    ag_in = [nc.dram_tensor(f"ag_in{l}", [16, NL], F32) for l in (1, 2)]
    ag_out = [nc.dram_tensor(f"ag_out{l}", [128, NL], F32, addr_space="Shared")
              for l in (1, 2)]
    ad_stage = [nc.dram_tensor(f"ad_stage{l}", [1, NL + NBW], BF16) for l in (1, 2)]
    # interleaved (wself | pself) bf16 pairs per node, 16 feature rows
    wp_stage = [nc.dram_tensor(f"wp_stage{l}", [16, 2 * (NL + 16)], BF16)
                for l in (1, 2)]

    with tile.TileContext(nc, num_cores=NC) as tc:
        with tc.tile_pool(name="const", bufs=1) as cpool:
            w1t = cpool.tile([128, KB, H], BF16)
            nc.sync.dma_start(out=w1t[:], in_=w1[:].rearrange("(b p) h -> p b h", p=128))
            w2t = cpool.tile([16, H], BF16)
            nc.sync.dma_start(out=w2t[:], in_=w2[:])
            a1t = cpool.tile([16, 16], BF16)
            nc.sync.dma_start(out=a1t[:], in_=a1rep[:])
            ad1t = cpool.tile([16, 16], BF16)
            nc.sync.dma_start(out=ad1t[:], in_=ad1rep[:])
            a2t = cpool.tile([16, 16], BF16)
            nc.sync.dma_start(out=a2t[:], in_=a2rep[:])
            ad2t = cpool.tile([16, 16], BF16)
            nc.sync.dma_start(out=ad2t[:], in_=ad2rep[:])
            b1t = cpool.tile([16, 1], F32)
            nc.sync.dma_start(out=b1t[:], in_=b1p[:])
            b2t = cpool.tile([16, 1], F32)
            nc.sync.dma_start(out=b2t[:], in_=b2p[:])
            selt = cpool.tile([128, 16], BF16)
            nc.sync.dma_start(out=selt[:], in_=selp[:])
            idt = cpool.tile([16, 16], BF16)
            nc.sync.dma_start(out=idt[:], in_=identp[:])
            zero16 = cpool.tile([1, NBW], BF16)
            nc.vector.memset(zero16[:], 0.0)

            # out1T and out2T share one slot: all out1T reads (phase A2)
            # complete before any out2T write (layer-2 edge phase).
            out1T = cpool.tile([16, NL2], BF16, tag="outT")
            out2T = cpool.tile([16, NL2], BF16, tag="outT")

            def phase_a_chunk(l, c, pa, pap, h_matmul):
                """Emit table-gen ops for chunk c of layer l."""
                sl = slice(c * CH, (c + 1) * CH)
                ph = pap.tile([16, CH], F32, tag="ph")
                h_matmul(ph, sl)
                hch = pa.tile([16, CH], BF16, tag="hch")
                nc.scalar.activation(hch[:], ph[:], Copy)
                at, adt = (a1t, ad1t) if l == 0 else (a2t, ad2t)
                pas = pap.tile([16, CH], F32, tag="pas")
                nc.tensor.matmul(pas[:], lhsT=at[:], rhs=hch[:], start=True, stop=True)
                pad_ = pap.tile([16, CH], F32, tag="pad")
                nc.tensor.matmul(pad_[:], lhsT=adt[:], rhs=hch[:], start=True, stop=True)
                packed = pa.tile([16, CH], F32, tag="packed")
                pb = packed[:].bitcast(BF16)
                nc.scalar.activation(pb[:, 0::2], hch[:], Copy)
                nc.scalar.activation(pb[:, 1::2], pas[:], Copy)
                nc.sync.dma_start(out=ag_in[l][:, sl], in_=packed[:])
                adfull = pa.tile([16, CH], BF16, tag="adfull")
                nc.scalar.activation(adfull[:], pad_[:], Copy)
                nc.sync.dma_start(out=ad_stage[l][0:1, sl], in_=adfull[0:1, :])
                # analytic self-loop contribution for this chunk
                tself = pa.tile([16, CH], F32, tag="tself")
                nc.vector.tensor_add(tself[:], pas[:], adfull[:])
                nc.scalar.activation(tself[:], tself[:], Prelu, alpha=NEG_SLOPE)
                nc.scalar.activation(tself[:], tself[:], Exp)
                wsp = pa.tile([16, 2 * CH], BF16, tag="wsp")
                nc.vector.tensor_mul(wsp[:, 0::2], tself[:], hch[:])
                nc.scalar.activation(wsp[:, 1::2], tself[:], Copy)
                nc.sync.dma_start(
                    out=wp_stage[l][:, 2 * c * CH:2 * (c + 1) * CH], in_=wsp[:])

            # ---------------- phase A (layer 1): tables from x^T ----------
            with (
                tc.tile_pool(name="pa", bufs=2) as pa,
                tc.tile_pool(name="pap", bufs=2, space="PSUM") as pap,
            ):
                def h1_matmul(ph, sl):
                    xt_t = pa.tile([128, KB, CH], BF16, tag="xt")
                    nc.sync.dma_start(
                        out=xt_t[:],
                        in_=xt[:, sl].rearrange("(b p) n -> p b n", p=128))
                    for b in range(KB):
                        nc.tensor.matmul(ph[:], lhsT=w1t[:, b, :], rhs=xt_t[:, b, :],
                                         start=(b == 0), stop=(b == KB - 1))
                for c in range(NCH):
                    phase_a_chunk(0, c, pa, pap, h1_matmul)
                nc.sync.dma_start(out=ad_stage[0][0:1, NL:], in_=zero16[:])

            def run_layer(l, writer, post_tile=None):
                nc.gpsimd.collective_compute(
                    "AllGather", mybir.AluOpType.bypass, replica_groups=rg,
                    ins=[ag_in[l][:]], outs=[ag_out[l][:]])
                with tc.tile_pool(name=f"tab{l}", bufs=1) as tabp:
                    table = tabp.tile([128, NLP], F32, tag="table")
                    nc.sync.dma_start(out=table[:, :NL], in_=ag_out[l][:])
                    nc.vector.memset(table[:, NL:], SENTINEL_PACKED)
                    with (
                        tc.tile_pool(name=f"st{l}", bufs=2) as st,
                        tc.tile_pool(name=f"ed{l}", bufs=2) as ed,
                        tc.tile_pool(name=f"sq{l}", bufs=2) as sq,
                        tc.tile_pool(name=f"edp{l}", bufs=2, space="PSUM") as edp,
                    ):
                        prev = None
                        for t in range(NT + 1):
                            cur = None
                            if t < NT:
                                # stream in + scatters/gather for tile t
                                it = st.tile([128, IW], I16, tag="it")
                                nc.sync.dma_start(out=it[:], in_=idxsp[:, t * IW:(t + 1) * IW])
                                mt = st.tile([128, T_e], BF16, tag="mt")
                                nc.sync.dma_start(out=mt[:], in_=maskp[:, t * T_e:(t + 1) * T_e])
                                adb = st.tile([128, NBW], BF16, tag="adb")
                                nc.sync.dma_start(
                                    out=adb[:],
                                    in_=ad_stage[l][0:1, t * NB: t * NB + NBW]
                                    .to_broadcast([128, NBW]))
                                sia = st.tile([128, 512], I16, tag="sia")
                                nc.sync.dma_start(out=sia[:], in_=sidxap[:, t * 512:(t + 1) * 512])
                                sib = st.tile([128, 512], I16, tag="sib")
                                nc.sync.dma_start(out=sib[:], in_=sidxbp[:, t * 512:(t + 1) * 512])
                                wpst = st.tile([16, 1024], BF16, tag="wpst")
                                nc.sync.dma_start(
                                    out=wpst[:],
                                    in_=wp_stage[l][:, 2 * t * NB: 2 * t * NB + 1024])
                                # alpha_dst at run starts (halves), then a
                                # masked scan forward-fills to every edge slot
                                half = T_e // 2
                                ads = ed.tile([128, T_e], BF16, tag="ads")
                                nc.gpsimd.local_scatter(
                                    ads[:, :half], adb[:, :512], sia[:],
                                    channels=128, num_elems=half, num_idxs=512)
                                nc.gpsimd.local_scatter(
                                    ads[:, half:], adb[:, :512], sib[:],
                                    channels=128, num_elems=half, num_idxs=512)
                                ghs = ed.tile([128, T_e], F32, tag="ghs")
                                nc.gpsimd.ap_gather(
                                    ghs[:], table[:], it[:, 0:T16],
                                    channels=128, num_elems=NLP, d=1, num_idxs=T_e)
                                cur = dict(it=it, mt=mt, wpst=wpst, ghs=ghs, ads=ads, t=t)

                            if prev is not None:
                                # tail of tile t-1: boundary gather, matmuls,
                                # normalize, write (+ interleaved phase-A2 work)
                                pt = prev["t"]
                                bg = ed.tile([128, 1024], BF16, tag="bg")
                                nc.gpsimd.ap_gather(
                                    bg[:].rearrange("p (n d) -> p n d", d=2),
                                    prev["sc"][:], prev["it"][:, T16:],
                                    channels=128, num_elems=T_e, d=2, num_idxs=512)
                                pq = edp.tile([16, 1024], F32, tag="pq")
                                for h0 in (0, 512):
                                    hs = slice(h0, h0 + 512)
                                    nc.tensor.matmul(pq[:, hs], lhsT=selt[:],
                                                     rhs=bg[:, hs],
                                                     start=True, stop=False)
                                    nc.tensor.matmul(pq[:, hs], lhsT=idt[:],
                                                     rhs=prev["wpst"][:, hs],
                                                     start=False, stop=True)
                                prev.update(bg=bg, pq=pq)

                            if t < NT:
                                # per-edge math for tile t: forward-fill ad,
                                # then z = as + ad -> prelu -> exp
                                ade = ed.tile([128, T_e], BF16, tag="ade")
                                nc.vector.tensor_tensor_scan(
                                    ade[:], cur["mt"][:], cur["ads"][:], 0.0,
                                    mybir.AluOpType.mult, mybir.AluOpType.add)
                                gb = cur["ghs"][:].bitcast(BF16)  # [128, 2*T_e]
                                z = ed.tile([128, T_e], BF16, tag="z")
                                nc.vector.tensor_add(z[:], gb[:, 1::2], ade[:])
                                nc.scalar.activation(z[:], z[:], Prelu, alpha=NEG_SLOPE)
                                pp = ed.tile([128, T_e], BF16, tag="pp")
                                nc.scalar.activation(pp[:], z[:], Exp)

                            if prev is not None:
                                # normalize + write t-1 (fills the DVE gap while
                                # ACT computes prelu/exp for tile t)
                                pt = prev["t"]
                                pq = prev["pq"]
                                pqv = pq[:].rearrange("p (n d) -> p n d", d=2)
                                rc = sq.tile([16, 512], F32, tag="rc")
                                nc.vector.reciprocal_approx_fast(out=rc[:], in_=pqv[:, :, 1])
                                uv = sq.tile([16, NB], F32, tag="uv")
                                nc.vector.tensor_mul(uv[:], pqv[:, :NB, 0], rc[:, :NB])
                                writer(pt, uv)
                                if post_tile is not None:
                                    post_tile(pt)

                            if t < NT:
                                ww = ed.tile([128, T_e], BF16, tag="ww")
                                nc.vector.tensor_mul(ww[:], pp[:], gb[:, 0::2])
                                sc = ed.tile([128, T_e, 2], BF16, tag="sc")
                                nc.vector.tensor_tensor_scan(
                                    sc[:, :, 0], cur["mt"][:], ww[:], 0.0,
                                    mybir.AluOpType.mult, mybir.AluOpType.add)
                                nc.vector.tensor_tensor_scan(
                                    sc[:, :, 1], cur["mt"][:], pp[:], 0.0,
                                    mybir.AluOpType.mult, mybir.AluOpType.add)
                                cur["sc"] = sc
                            prev = cur

            def write1(t, uv):
                nc.scalar.activation(out1T[:, t * NB:(t + 1) * NB], uv[:],
                                     Relu, bias=b1t[:, 0:1])

            def write2(t, uv):
                nc.scalar.activation(out2T[:, t * NB:(t + 1) * NB], uv[:],
                                     Ident, bias=b2t[:, 0:1])

            # phase A2 (layer-2 tables) interleaved into the layer-1 loop:
            # chunk c reads out1T columns written by edge tile c.
            with (
                tc.tile_pool(name="pa2", bufs=1) as pa2,
                tc.tile_pool(name="pap2", bufs=1, space="PSUM") as pap2,
            ):
                def a2_chunk(c):
                    def h2_matmul(ph, sl):
                        nc.tensor.matmul(ph[:], lhsT=w2t[:], rhs=out1T[:, sl],
                                         start=True, stop=True)
                    phase_a_chunk(1, c, pa2, pap2, h2_matmul)
                    if c == NCH - 1:
                        nc.sync.dma_start(out=ad_stage[1][0:1, NL:], in_=zero16[:])

                run_layer(0, write1, post_tile=a2_chunk)

            if NL2 > NL:
                nc.vector.memset(out2T[:, NL:], 0.0)
            run_layer(1, write2)

            # ---------------- log_softmax + transpose + store -------------
            with (
                tc.tile_pool(name="fin", bufs=2) as fin,
                tc.tile_pool(name="finp", bufs=4, space="PSUM") as finp,
                tc.tile_pool(name="fino", bufs=1) as fino,
            ):
                nodemaj = fino.tile([128, NBLK, H], F32, tag="nodemaj")
                for j in range(NBLK):
                    ptp = finp.tile([128, 16], BF16, tag="ptp")
                    nc.tensor.transpose(ptp[:], out2T[:, j * 128:(j + 1) * 128], idt[:])
                    nc.vector.tensor_copy(nodemaj[:, j, :], ptp[:, :H])
                mx = fin.tile([128, NBLK], F32, tag="mx")
                nc.vector.tensor_reduce(mx[:], nodemaj[:], axis=mybir.AxisListType.X,
                                        op=mybir.AluOpType.max)
                zz = fino.tile([128, NBLK, H], F32, tag="zz")
                nc.vector.tensor_sub(zz[:], nodemaj[:],
                                     mx[:, :, None].to_broadcast([128, NBLK, H]))
                es = fino.tile([128, NBLK, H], F32, tag="es")
                nc.scalar.activation(es[:], zz[:], Exp)
                sm = fin.tile([128, NBLK], F32, tag="sm")
                nc.vector.tensor_reduce(sm[:], es[:], axis=mybir.AxisListType.X,
                                        op=mybir.AluOpType.add)
                ls = fin.tile([128, NBLK], F32, tag="ls")
                nc.scalar.activation(ls[:], sm[:], mybir.ActivationFunctionType.Ln)
                outf = fino.tile([128, NBLK, H], F32, tag="outf")
                nc.vector.tensor_sub(outf[:], zz[:],
                                     ls[:, :, None].to_broadcast([128, NBLK, H]))
                nc.sync.dma_start(out=outp[:].rearrange("p (b h) -> p b h", h=H),
                                  in_=outf[:])

    nc.compile()
    if split:
        split_waits(nc, max_waits=max_waits, ctrl_max_waits=ctrl_max_waits)
    return nc


CTRL_TYPES = ("InstDrain", "InstNoOp", "InstHalt", "InstEventSemaphore")


def split_waits(nc, max_waits=2, ctrl_max_waits=1):
    """walrus in this container caps sync-waits per instruction; move excess
    waits onto preceding same-engine NoOps (each carrying one wait)."""
    for f in nc.m.functions:
        for bb in f.blocks:
            new_insts, changed = [], False
            for ins in bb.instructions:
                si = ins.sync_info
                cap = (ctrl_max_waits if type(ins).__name__ in CTRL_TYPES
                       else max_waits)
                if si is not None and si.on_wait is not None and len(si.on_wait) > cap:
                    waits = list(si.on_wait)
                    excess, keep = waits[:-cap] if cap else waits, waits[-cap:] if cap else []
                    for i, w in enumerate(excess):
                        nop = mybir.InstNoOp(name=f"{ins.name}-ws{i}", ins=[], outs=[])
                        nop.engine = ins.engine
                        nop.sync_info = mybir.SyncInfo(on_wait=[w], on_update=[])
                        new_insts.append(nop)
                    si.on_wait = keep
                    changed = True
                new_insts.append(ins)
            if changed:
                bb.instructions = new_insts
    # verify the rewrite stuck (pyo3 lists can copy-on-read)
    for f in nc.m.functions:
        for bb in f.blocks:
            for ins in bb.instructions:
                si = ins.sync_info
                cap = (ctrl_max_waits if type(ins).__name__ in CTRL_TYPES
                       else max_waits)
                assert si is None or si.on_wait is None or len(si.on_wait) <= cap, \
                    f"{ins.name}: {len(si.on_wait)} waits > {cap}"


# ------------------------------------------------------------ input packing

def make_in_maps(inputs, cfg, per_core, perm):
    NC, NL, H, F = cfg["NCORES"], cfg["NL"], cfg["H"], cfg["F"]
    BF = ml_dtypes.bfloat16
    x = np.asarray(inputs["x"], dtype=np.float32)
    xt_full = x.T.astype(BF)  # [F, N] bf16
    xt_perm = np.empty_like(xt_full)
    xt_perm[:, perm] = xt_full
    xt_full = xt_perm
    sel = np.zeros((128, 16), dtype=BF)
    sel[np.arange(128), np.arange(128) % 16] = 1.0

    def rep16(v):
        return np.ascontiguousarray(
            np.repeat(np.asarray(v, np.float32)[:, None], 16, 1).astype(BF))

    shared = {
        "w1": np.ascontiguousarray(np.asarray(inputs["W1"], np.float32).astype(BF)),
        "w2": np.ascontiguousarray(np.asarray(inputs["W2"], np.float32).astype(BF)),
        "a1rep": rep16(inputs["a_src1"]),
        "ad1rep": rep16(inputs["a_dst1"]),
        "a2rep": rep16(inputs["a_src2"]),
        "ad2rep": rep16(inputs["a_dst2"]),
        "b1p": np.ascontiguousarray(np.asarray(inputs["b1"], np.float32)[:, None]),
        "b2p": np.ascontiguousarray(np.asarray(inputs["b2"], np.float32)[:, None]),
        "selp": sel,
        "identp": np.eye(16, dtype=BF),
    }
    in_maps = []
    for c in range(NC):
        m = dict(shared)
        m["xt"] = np.ascontiguousarray(xt_full[:, c * NL:(c + 1) * NL])
        m["idxs"] = per_core[c]["idxs"]
        m["mask"] = per_core[c]["mask"]
        m["sidxa"] = per_core[c]["sidxa"]
        m["sidxb"] = per_core[c]["sidxb"]
        in_maps.append(m)
    return in_maps


def unshard_output(results, cfg, perm):
    NC, NL, H = cfg["NCORES"], cfg["NL"], cfg["H"]
    NBLK = math.ceil(NL / 128)
    parts = []
    for c in range(NC):
        a = np.asarray(results[c]["out"]).reshape(128, NBLK, H)
        a = a.transpose(1, 0, 2).reshape(NBLK * 128, H)[:NL]
        parts.append(a)
    out_new = np.concatenate(parts, axis=0)
    return out_new[perm]


# ------------------------------------------------------------------- driver

_CACHE = {}


def run_on_hw(inputs, cfg, trace=False, tmpdir=None):
    import os
    import shutil
    from concourse.bass_utils import run_bass_kernel_spmd
    if tmpdir is not None and os.path.isdir(tmpdir):
        shutil.rmtree(tmpdir, ignore_errors=True)
    if tmpdir is not None:
        os.makedirs(tmpdir, exist_ok=True)
    perm = balance_permutation(inputs["edge_index"], cfg)
    T_e, per_core = host_prep(inputs["edge_index"], cfg, perm)
    key = (cfg["N"], T_e)
    if key not in _CACHE:
        _CACHE[key] = build_nc(cfg, T_e)
    nc = _CACHE[key]
    in_maps = make_in_maps(inputs, cfg, per_core, perm)
    res = run_bass_kernel_spmd(nc, in_maps, list(range(cfg["NCORES"])),
                               trace=trace, tmpdir=tmpdir)
    out = unshard_output(res.results, cfg, perm)
    return out, res


def kernel(**inputs):
    out, _ = run_on_hw(inputs, FULL_CFG)
    return out.astype(np.float32)
